# revision 55
# baseline (speedup 1.0000x reference)
"""Trainium2 Bass kernel for nn_Attention_81037442941065.

Dual-attention module (spatial [b,h,n,n] + channel [b,h,d,d]) with
B=2, N=2048, DIM=1024, 16 heads of d=64.

Sharding: 8 cores = (2 batches) x (4 head-groups of 4 heads).
Each core computes its batch/head-group slice end-to-end and produces a
partial (over head groups) output projection; the host sums the 4 group
partials per batch (the "all-reduce after to_out") and adds b_out.

Schedule (v2): the wall-clock pole is ScalarE's exp stream (128
ACTIVATEs of [128,1024], ~1.39us each, ~178us total).  Everything else
is arranged around keeping that stream gapless from as early as
possible:
  - only z1T/yhT (the S operands) are computed before the spatial loop;
  - xh, z2 + channel-attn logits, channel softmax, out2 and the final
    projection all run as an "aux" stream drained into the PE's idle
    slots inside the spatial loop (one matmul per drain slot, anchored
    to the S matmul of that slot so the scheduler cannot hoist them);
  - x is DMA'd in token-column blocks (host pre-blocks it) so each
    xh chunk only needs its own 256KB slice, letting AV consume
    xh_aug[j] within microseconds of spatial start;
  - output is written bf16 (host accumulates fp32) to halve the tail
    DMA; softmax denominators use reciprocal_approx_fast (~5x faster
    than InstReciprocal at ~18 correct bits).

Dtypes: all matmul operands bf16 (fp32 accumulation in PSUM); softmax
statistics fp32.  End-to-end relative error ~4e-3 vs fp32 reference.

Per-core layouts (everything "T" is [channels, tokens]):
  z1T, yhT   : 2 tiles [128, 2048]  (head h at rows 64*(h%2) of tile h//2)
  xh_aug     : 16 tiles [128, 260] (per 128-token chunk; per head 65
               cols = 64 channels + a ones column so the AV matmul also
               produces the softmax denominators)
  spatial    : S^T = yh @ z1^T computed [keys, queries]; the two heads
               of a pair run as concurrent PE row-tiles (base partition
               0/64); exp on ScalarE (scale 1/8 fused, no max
               subtraction - logits are small); AV matmul lhsT=[xh|1]
               accumulates over key chunks -> rows 0..63 =
               unnormalized out1^T, row 64 = sum of exp.
  channel    : logits accumulated per token-chunk into an SBUF fp32
               accumulator (PSUM stays free for the spatial loop);
               softmax via Exp+accum_out and per-partition reciprocal.
"""

import sys

for _p in ("/opt/trn_rl_repo", "/opt/pypackages"):
    if _p not in sys.path:
        sys.path.insert(0, _p)

import ml_dtypes
import numpy as np
from contextlib import ExitStack

import concourse.bacc as bacc
import concourse.mybir as mybir
import concourse.tile as tile
from concourse.tile import add_dep_helper
from concourse.bass_utils import run_bass_kernel_spmd

F32 = mybir.dt.float32
BF16 = mybir.dt.bfloat16
ATT = mybir.dt.bfloat16   # attention-internal matmul dtype
F8 = mybir.dt.float8e4    # e4m3: AV operands (P in (0,7.4], xh ~N(0,0.4))
DR = mybir.MatmulPerfMode.DoubleRow
EXP = mybir.ActivationFunctionType.Exp
COPY = mybir.ActivationFunctionType.Copy
XH8_H = 80                # fp8 xh head stride (16B-aligned for DoubleRow)
XH8_C = 4 * XH8_H         # fp8 xh chunk stride

B, N, DIM = 2, 2048, 1024
HEADS, DH = 16, 64
G = 4              # head groups == cores per batch
HG = HEADS // G    # heads per group (4)
CIN = HG * DH      # inner channels per core (256)
NCORES = 8
KC = DIM // 128    # contraction chunks for projections (8)
NCH = N // 128     # 128-token chunks (16)
SCALE = DH ** -0.5            # 1/8
CM_SCALE = SCALE / (N / DH)   # 1/256


def _build_program():
    nc = bacc.Bacc(
        "TRN2", target_bir_lowering=False, debug=False, num_devices=NCORES
    )

    # ---- DRAM I/O ----
    # xB is x^T re-blocked host-side: xB[i*128+p, k*128+j] = x^T[k*128+p,
    # i*128+j], so each token-chunk's projection operand is one contiguous
    # [128, 1024] DMA.
    xB_d = nc.dram_tensor("xB", [N, DIM], BF16, kind="ExternalInput").ap()
    yT_d = nc.dram_tensor("yT", [DIM, N], BF16, kind="ExternalInput").ap()
    zT_d = nc.dram_tensor("zT", [DIM, N], BF16, kind="ExternalInput").ap()
    # weights are host-blocked to [128, KC*CIN]: wB[p, k*CIN+o] = w[k*128+p, o]
    # so each weight matrix is a single contiguous DMA
    wsa1_d = nc.dram_tensor("w_sa1", [128, KC * CIN], BF16,
                            kind="ExternalInput").ap()
    wsa2_d = nc.dram_tensor("w_sa2", [128, KC * CIN], BF16,
                            kind="ExternalInput").ap()
    wse1_d = nc.dram_tensor("w_se1", [128, KC * CIN], BF16,
                            kind="ExternalInput").ap()
    wse2_d = nc.dram_tensor("w_se2", [128, KC * CIN], BF16,
                            kind="ExternalInput").ap()
    wout_d = nc.dram_tensor("w_out", [CIN, DIM], ATT, kind="ExternalInput").ap()
    outT_d = nc.dram_tensor("outT", [DIM, N], ATT, kind="ExternalOutput").ap()

    with tile.TileContext(nc) as tc, ExitStack() as ctx:
        ppool = ctx.enter_context(tc.tile_pool(name="persist", bufs=1))

        # Persistent tiles.
        z1T = [ppool.tile([128, N], ATT, tag=f"z1T{m}", name=f"z1T{m}")
               for m in range(2)]
        yhT = [ppool.tile([128, N], ATT, tag=f"yhT{m}", name=f"yhT{m}")
               for m in range(2)]
        xh_aug = [ppool.tile([128, HG * (DH + 1)], ATT, tag=f"xa{i}",
                             name=f"xa{i}") for i in range(NCH)]
        secm_sb = [ppool.tile([128, DH], ATT, tag=f"cm{p}", name=f"cm{p}")
                   for p in range(2)]
        rs = [ppool.tile([64, 1], F32, tag=f"rs{h}", name=f"rs{h}")
              for h in range(HG)]
        rcm = [ppool.tile([64, 1], F32, tag=f"rcm{h}", name=f"rcm{h}")
               for h in range(HG)]
        cmacc = ppool.tile([64, HG * DH], F32, tag="cmacc", name="cmacc")

        # inputs (weights as single blocked tiles)
        wsa1_t = ppool.tile([128, KC * CIN], BF16, tag="wsa1", name="wsa1")
        wsa2_t = ppool.tile([128, KC * CIN], BF16, tag="wsa2", name="wsa2")
        wse1_t = ppool.tile([128, KC * CIN], BF16, tag="wse1", name="wse1")
        wse2_t = ppool.tile([128, KC * CIN], BF16, tag="wse2", name="wse2")
        zTt = [ppool.tile([128, N], BF16, tag=f"z{k}", name=f"z{k}")
               for k in range(KC)]
        yTt = [ppool.tile([128, N], BF16, tag=f"y{k}", name=f"y{k}")
               for k in range(KC)]
        xcol = [ppool.tile([128, DIM], BF16, tag=f"xc{i}", name=f"xc{i}")
                for i in range(NCH)]
        wq = [ppool.tile([64, DIM], ATT, tag=f"wq{q}", name=f"wq{q}")
              for q in range(HG)]
        cat4 = [ppool.tile([64, N], ATT, tag=f"cat{h}", name=f"cat{h}")
                for h in range(HG)]

        ptpool = ctx.enter_context(tc.tile_pool(name="pt", bufs=4))
        tpool = ctx.enter_context(tc.tile_pool(name="tails", bufs=3))
        opool = ctx.enter_context(tc.tile_pool(name="oout", bufs=4))
        z2pool = ctx.enter_context(tc.tile_pool(name="z2s", bufs=3))

        # ---- All input DMAs on the sync queue in strict priority order:
        # wire order == need order (wsa/z/y gate the exp-stream start; wse/x
        # feed the pass-0 aux stream; wq is needed only from pass 4).
        # Scalar stays clean so z1T/yhT PSUM copies aren't queued behind
        # DGE ring waits.
        nc.sync.dma_start(wsa1_t[:], wsa1_d[:, :])
        nc.sync.dma_start(wsa2_t[:], wsa2_d[:, :])
        for k in range(KC):
            nc.sync.dma_start(zTt[k][:], zT_d[k * 128:(k + 1) * 128, :])
        for k in range(KC):
            nc.sync.dma_start(yTt[k][:], yT_d[k * 128:(k + 1) * 128, :])
        nc.sync.dma_start(wse1_t[:], wse1_d[:, :])
        nc.sync.dma_start(wse2_t[:], wse2_d[:, :])
        for i in range(NCH):
            nc.sync.dma_start(xcol[i][:], xB_d[i * 128:(i + 1) * 128, :])
        for q in range(HG):
            nc.sync.dma_start(wq[q][:], wout_d[q * 64:(q + 1) * 64, :])

        # constants on gpsimd (ones columns needed by the first AV)
        nc.gpsimd.memset(cmacc[:], 0.0)
        for i in range(NCH):
            dst = xh_aug[i][:].rearrange("p (h c) -> p h c", c=DH + 1)
            nc.gpsimd.memset(dst[:, :, DH:DH + 1], 1.0)

        # PE warmup: HAM boots at 1.2 GHz and needs ~3.4us of sustained
        # matmul activity to unthrottle.  Burn that in on a zeroed tile
        # before the first real operand lands so the projections run at
        # 2.4 GHz from the start.
        warm0 = ppool.tile([128, 512], BF16, tag="warm0", name="warm0")
        nc.vector.memset(warm0[:], 0.0)

        # cat4 accumulates out1 (tails) and out2 (aux adds) in either order
        for h in range(HG):
            nc.vector.memset(cat4[h][:], 0.0)

        # ============ Pre-spatial: z1T / yhT projections only ============
        # k-major emission across all 8 (m, nb) chains: each arriving
        # zTt[k]/yTt[k] DMA tile unlocks 8 consecutive matmuls, so the PE
        # tracks the DMA feed rate instead of head-of-line-blocking on one
        # chain's next k-tile.  bufs=8 = the whole PSUM (spatial pools open
        # after this scope closes).
        with tc.tile_pool(name="psp", bufs=8, space="PSUM") as psp:
            pw = psp.tile([128, 512], F32, tag="pj", name="pwarm")
            for w in range(20):
                nc.tensor.matmul(pw[:], lhsT=warm0[:, 0:128], rhs=warm0[:],
                                 start=(w == 0), stop=(w == 19))
            for rnd, (dst, wt, srcs) in enumerate(
                    ((z1T, wsa1_t, zTt), (yhT, wsa2_t, yTt))):
                pss = {}
                for m in range(2):
                    for nb in range(4):
                        pss[(m, nb)] = psp.tile([128, 512], F32, tag="pj",
                                                name=f"ps{rnd}{m}{nb}")
                for k in range(KC):
                    for m in range(2):
                        for nb in range(4):
                            nc.tensor.matmul(
                                pss[(m, nb)][:],
                                lhsT=wt[:, k * CIN + m * 128:
                                        k * CIN + (m + 1) * 128],
                                rhs=srcs[k][:, nb * 512:(nb + 1) * 512],
                                start=(k == 0), stop=(k == KC - 1),
                            )
                # m=0 copies first (the first spatial pass reads only m=0),
                # split across scalar and vector so the copy tail halves;
                # all m=1 copies go to vector so they can't delay the first
                # exps behind them in the scalar queue
                for m in range(2):
                    for nb in range(4):
                        src_ps = pss[(m, nb)][:]
                        dslice = dst[m][:, nb * 512:(nb + 1) * 512]
                        if m == 0 and nb % 2 == 0:
                            nc.scalar.copy(dslice, src_ps)
                        else:
                            nc.vector.tensor_copy(dslice, src_ps)

        # ============ Spatial loop with full aux stream ============
        # PSUM: S 2x[128,1024] (4 banks) + av 2x[128,512] (2 banks) +
        # aux 2x[128,512] (2 banks) = 8 banks exactly.
        with tc.tile_pool(name="psS", bufs=2, space="PSUM") as psS, \
             tc.tile_pool(name="psAV", bufs=2, space="PSUM") as psAV, \
             tc.tile_pool(name="psaux", bufs=2, space="PSUM") as psaux:

            # Aux matmul stream: xh / z2+channel-logits / out2 / final
            # projection, one PE instruction per thunk, drained inside the
            # spatial j-loops so the PE always has ready work while ScalarE
            # runs the exps.
            aux_thunks = []
            final_psf = {}
            xh_ps = {}
            z2_ps = {}
            cm_ps = {}
            z2n_t = {}

            def emit_xh_mm(i, k):
                if k == 0:
                    xh_ps[i] = psaux.tile([128, 512], F32, tag="aux",
                                          name=f"psx{i}")
                ps = xh_ps[i]
                mm = nc.tensor.matmul(
                    ps[:, 0:CIN],
                    lhsT=xcol[i][:, k * 128:(k + 1) * 128],
                    rhs=wse1_t[:, k * CIN:(k + 1) * CIN],
                    start=(k == 0), stop=(k == KC - 1),
                )
                if k == KC - 1:
                    src = ps[:, 0:CIN].rearrange("p (h c) -> p h c", c=DH)
                    dst = xh_aug[i][:].rearrange("p (h c) -> p h c", c=DH + 1)
                    nc.vector.tensor_copy(dst[:, :, 0:DH], src)
                    del xh_ps[i]
                return mm

            def emit_z2_mm(i, k):
                if k == 0:
                    z2_ps[i] = psaux.tile([128, 512], F32, tag="aux",
                                          name=f"psz2_{i}")
                ps = z2_ps[i]
                mm = nc.tensor.matmul(
                    ps[:, 0:CIN],
                    lhsT=zTt[k][:, i * 128:(i + 1) * 128],
                    rhs=wse2_t[:, k * CIN:(k + 1) * CIN],
                    start=(k == 0), stop=(k == KC - 1),
                )
                if k == KC - 1:
                    z2n = z2pool.tile([128, CIN], ATT, tag="z2n",
                                      name=f"z2n{i}")
                    nc.vector.tensor_copy(z2n[:], ps[:, 0:CIN])
                    z2n_t[i] = z2n
                    del z2_ps[i]
                return mm

            def emit_cm_mm(i, h):
                if h == 0:
                    cm_ps[i] = psaux.tile([128, 512], F32, tag="aux",
                                          name=f"pscm{i}")
                ps = cm_ps[i]
                mm = nc.tensor.matmul(
                    ps[0:64, h * DH:(h + 1) * DH],
                    lhsT=xh_aug[i][:, 65 * h:65 * h + DH],
                    rhs=z2n_t[i][:, DH * h:DH * (h + 1)],
                    start=True, stop=True,
                )
                if h == HG - 1:
                    nc.vector.tensor_add(cmacc[:], ps[0:64, 0:HG * DH],
                                         cmacc[:])
                    del cm_ps[i]
                    del z2n_t[i]
                    if i == NCH - 1:
                        # channel-attn softmax, DMA'd into pair-packed secm_sb
                        for hh in range(HG):
                            p_, off = hh // 2, 64 * (hh % 2)
                            st = z2pool.tile([64, DH], ATT, tag="cmstage",
                                             name=f"cmstage{hh}")
                            nc.scalar.activation(
                                st[:], cmacc[:, hh * DH:(hh + 1) * DH], EXP,
                                scale=CM_SCALE, accum_out=rs[hh][0:64, 0:1])
                            nc.vector.reciprocal(rcm[hh][0:64, 0:1],
                                                 rs[hh][0:64, 0:1])
                            nc.vector.tensor_scalar_mul(st[:], st[:],
                                                        rcm[hh][0:64, 0:1])
                            nc.sync.dma_start(secm_sb[p_][off:off + 64, :],
                                              st[:])
                return mm

            def emit_out2(h, nb):
                p_, off = h // 2, 64 * (h % 2)
                pso = psaux.tile([128, 512], F32, tag="aux",
                                 name=f"pso{h}{nb}")
                mm = nc.tensor.matmul(
                    pso[0:64, :],
                    lhsT=secm_sb[p_][off:off + 64, :],
                    rhs=yhT[p_][off:off + 64, nb * 512:(nb + 1) * 512],
                    start=True, stop=True,
                )
                dst = cat4[h][:, nb * 512:(nb + 1) * 512]
                nc.vector.tensor_add(dst, pso[0:64, :], dst)
                return mm

            def emit_final_mm(d, nb, q):
                if q == 0:
                    if nb == 3:
                        # the spatial loop is over by nb=3: use the freed
                        # 4-bank psS pool (two d-chains per [128,1024] tile)
                        # so chains pipeline instead of stalling on the
                        # 2-bank aux rotation
                        if d % 2 == 0:
                            final_psf[(d // 2, "p3")] = psS.tile(
                                [128, 1024], F32, tag="S", name=f"psf3_{d}")
                        big = final_psf[(d // 2, "p3")]
                        final_psf[(d, nb)] = big[:, (d % 2) * 512:
                                                 (d % 2) * 512 + 512]
                    else:
                        final_psf[(d, nb)] = psaux.tile(
                            [128, 512], F32, tag="aux", name=f"psf{d}{nb}")[:]
                psf = final_psf[(d, nb)]
                mm = nc.tensor.matmul(
                    psf,
                    lhsT=wq[q][:, d * 128:(d + 1) * 128],
                    rhs=cat4[q][:, nb * 512:(nb + 1) * 512],
                    start=(q == 0), stop=(q == HG - 1),
                )
                if q == HG - 1:
                    ob = opool.tile([128, 512], ATT, tag="ob",
                                    name=f"ob{d}{nb}")
                    if nb == 3:
                        nc.scalar.copy(ob[:], psf)
                    else:
                        nc.vector.tensor_copy(ob[:], psf)
                    nc.sync.dma_start(
                        outT_d[d * 128:(d + 1) * 128,
                               nb * 512:(nb + 1) * 512],
                        ob[:],
                    )
                return mm

            # static aux queue: all xh chunks, then z2+cm per chunk, then
            # out2; finals are appended as their cat4 blocks complete
            for i in range(NCH):
                for k in range(KC):
                    aux_thunks.append(lambda i=i, k=k: emit_xh_mm(i, k))
            for i in range(NCH):
                for k in range(KC):
                    aux_thunks.append(lambda i=i, k=k: emit_z2_mm(i, k))
                for h in range(HG):
                    aux_thunks.append(lambda i=i, h=h: emit_cm_mm(i, h))
            for h in range(HG):
                for nb in range(4):
                    aux_thunks.append(lambda h=h, nb=nb: emit_out2(h, nb))

            def queue_finals(nb, ds=range(8)):
                for d in ds:
                    for q in range(HG):
                        aux_thunks.append(
                            lambda d=d, nb=nb, q=q: emit_final_mm(d, nb, q))

            def drain_aux(k, anchor=None):
                # anchor pins the aux matmul into this drain slot's position
                # in the PE stream - the scheduler's gap-filler otherwise
                # hoists thunks into earlier windows where their inputs are
                # still several microseconds from ready
                for _ in range(k):
                    if aux_thunks:
                        mm = aux_thunks.pop(0)()
                        if anchor is not None and mm is not None:
                            add_dep_helper(mm.ins, anchor.ins, sync=False,
                                           reason="pin aux to drain slot")

            # drains per j-slot for each pass (pass = 2*ib + p_): front-load
            # xh (consumed by AV from pass 0) and z2/cm, then pace the
            # remaining 128 aux matmuls so no pass runs dry (HAM re-throttles
            # the PE clock if it idles)
            DRAIN_SCHED = [8, 4, 4, 4, 2, 2, 2, 2]

            def make_tail(p_, ib, avs, ptt_last):
                # Two-part tail.  Head (next iteration, j==0): the last
                # j-pair's AV matmuls, the avsb copies that release the AV
                # PSUM banks, and a small DMA that spreads each denominator
                # row [1,512] to [64,8] so its reciprocal is ~150ns on DVE
                # instead of a 3.3us FIFO-hogging [1,512] InstReciprocal.
                # Norm (j==2): reciprocal, DMA back, broadcast, scale, add.
                icol = ib * 512
                avsbs, d64s = [], []

                def emit_head():
                    for hh in range(2):
                        h = 2 * p_ + hh
                        nc.tensor.matmul(
                            avs[hh][0:DH + 1, :],
                            lhsT=xh_aug[NCH - 1][:, 65 * h:65 * h + DH + 1],
                            rhs=ptt_last[:, 512 * hh:512 * hh + 512],
                            start=False, stop=True,
                        )
                    for hh in range(2):
                        avsb = tpool.tile([DH + 1, 512], F32, tag="avsb",
                                          name=f"avsb{p_}{ib}{hh}")
                        nc.vector.tensor_copy(avsb[:], avs[hh][0:DH + 1, :])
                        avsbs.append(avsb)
                        if ib != 3:
                            d64 = tpool.tile([64, 8], F32, tag="d64",
                                             name=f"d64_{p_}{ib}{hh}")
                            nc.sync.dma_start(d64[:], avsb[DH:DH + 1, :])
                            d64s.append(d64)

                def emit_norm():
                    for hh in range(2):
                        h = 2 * p_ + hh
                        rc = tpool.tile([1, 512], F32, tag="rc",
                                        name=f"rc{p_}{ib}{hh}")
                        if ib == 3:
                            # latency-critical last tails: direct reciprocal
                            # beats the two-DMA-hop partition-spread version
                            nc.vector.reciprocal(rc[:],
                                                 avsbs[hh][DH:DH + 1, :])
                        else:
                            d64r = tpool.tile([64, 8], F32, tag="d64r",
                                              name=f"d64r{p_}{ib}{hh}")
                            nc.vector.reciprocal(d64r[:], d64s[hh][:])
                            nc.sync.dma_start(rc[:], d64r[:])
                        bc = tpool.tile([64, 512], F32, tag="bc",
                                        name=f"bc{p_}{ib}{hh}")
                        nc.gpsimd.partition_broadcast(bc[:], rc[:])
                        tmp = tpool.tile([64, 512], F32, tag="tmp",
                                         name=f"tmp{p_}{ib}{hh}")
                        nc.vector.tensor_mul(tmp[:], avsbs[hh][0:DH, :], bc[:])
                        dst = cat4[h][:, icol:icol + 512]
                        nc.vector.tensor_add(dst, tmp[:], dst)
                return emit_head, emit_norm

            pending_tail = None
            # --- spatial attention: iterations (ib 512-block, pair),
            #     processing key chunks two at a time (j-pairs) ---
            for ib in range(4):
                for p_ in range(2):
                    # nb's cat4 block is complete once BOTH pairs' tails ran;
                    # the second pair's tails execute during (ib+1, p0), so
                    # finals(nb) join the aux queue at (ib+1, p1)
                    if p_ == 1 and ib >= 1:
                        queue_finals(ib - 1)
                    icol = ib * 512
                    ndrain = DRAIN_SCHED[2 * ib + p_]
                    avs = [psAV.tile([128, 512], F32, tag="av",
                                     name=f"av{p_}{ib}{q}") for q in range(2)]
                    pair_t = [None] * NCH
                    for j in range(NCH):  # key chunks
                        spt = psS.tile([128, 1024], F32, tag="S",
                                       name=f"S{p_}{ib}{j}")
                        s_anchor = None
                        for hh in range(2):
                            off = 64 * hh
                            s_anchor = nc.tensor.matmul(
                                spt[:, 512 * hh:512 * hh + 512],
                                lhsT=yhT[p_][off:off + 64,
                                             j * 128:(j + 1) * 128],
                                rhs=z1T[p_][off:off + 64, icol:icol + 512],
                                start=True, stop=True,
                            )
                        ptt = ptpool.tile([128, 1024], ATT, tag="pt",
                                          name=f"pt{p_}{ib}{j}")
                        nc.scalar.activation(ptt[:], spt[:], EXP, scale=SCALE)
                        pair_t[j] = ptt
                        if pending_tail is not None:
                            if j == 0:
                                pending_tail[0]()
                            elif j == 2:
                                pending_tail[1]()
                                pending_tail = None
                        drain_aux(ndrain, s_anchor)
                        if j > 0:
                            for hh in range(2):
                                h = 2 * p_ + hh
                                nc.tensor.matmul(
                                    avs[hh][0:DH + 1, :],
                                    lhsT=xh_aug[j - 1][:, 65 * h:65 * h + DH + 1],
                                    rhs=pair_t[j - 1][:, 512 * hh:512 * hh + 512],
                                    start=(j == 1), stop=False,
                                )
                    pending_tail = make_tail(p_, ib, avs, pair_t[NCH - 1])
            pending_tail[0]()
            pending_tail[1]()
            # warm-keeper: the last tail's normalization chain
            # (dma->recip->dma->broadcast->mul->add) leaves the PE idle just
            # long enough for HAM to re-throttle the clock to 1.2 GHz right
            # before the last 32 final-projection matmuls.  Keep it busy
            # with throwaway matmuls whose results are never read.
            warm = psaux.tile([128, 512], F32, tag="aux", name="warmk")
            for w in range(40):
                nc.tensor.matmul(
                    warm[:],
                    lhsT=yhT[0][0:128, 0:128],
                    rhs=z1T[0][0:128, 0:512],
                    start=(w == 0), stop=(w == 39),
                )
            queue_finals(3)
            drain_aux(len(aux_thunks))

    nc.compile()
    return nc


_NC_CACHE = {}


def _get_program():
    if "nc" not in _NC_CACHE:
        _NC_CACHE["nc"] = _build_program()
    return _NC_CACHE["nc"]


def _prep_input_maps(x, y, z, w_sa1, w_sa2, w_se1, w_se2, w_out):
    bf16 = lambda a: np.ascontiguousarray(
        np.asarray(a, dtype=np.float32).astype(ml_dtypes.bfloat16))
    # wB[p, k*CIN+o] = w[k*128+p, o]
    wblk = lambda w: w.reshape(KC, 128, CIN).transpose(1, 0, 2) \
                      .reshape(128, KC * CIN)
    maps = []
    for c in range(NCORES):
        b, g = divmod(c, G)
        sl = slice(g * CIN, (g + 1) * CIN)
        xT = np.asarray(x)[b].T  # [DIM, N]
        # xB[i*128+p, k*128+j] = xT[k*128+p, i*128+j]
        xBlk = xT.reshape(KC, 128, NCH, 128).transpose(2, 1, 0, 3) \
                 .reshape(N, DIM)
        maps.append({
            "xB": bf16(xBlk),
            "yT": bf16(np.asarray(y)[b].T),
            "zT": bf16(np.asarray(z)[b].T),
            "w_sa1": bf16(wblk(np.asarray(w_sa1)[:, sl])),
            "w_sa2": bf16(wblk(np.asarray(w_sa2)[:, sl])),
            "w_se1": bf16(wblk(np.asarray(w_se1)[:, sl])),
            "w_se2": bf16(wblk(np.asarray(w_se2)[:, sl])),
            "w_out": bf16(np.asarray(w_out)[sl, :]),
        })
    return maps


def run(inputs, trace=False, trace_kwargs=None):
    """Run on hardware; returns (full_output, BassKernelResults)."""
    nc = _get_program()
    in_maps = _prep_input_maps(
        inputs["x"], inputs["y"], inputs["z"],
        inputs["w_sa1"], inputs["w_sa2"], inputs["w_se1"], inputs["w_se2"],
        inputs["w_out"],
    )
    res = run_bass_kernel_spmd(
        nc, in_maps, list(range(NCORES)), trace=trace,
        trace_kwargs=trace_kwargs or {},
    )
    out = np.zeros((B, N, DIM), dtype=np.float32)
    for c in range(NCORES):
        b, _g = divmod(c, G)
        out[b] += np.asarray(res.results[c]["outT"]).astype(np.float32).T
    out += np.asarray(inputs["b_out"], dtype=np.float32)
    return out, res


def kernel(**inputs) -> np.ndarray:
    out, _ = run(inputs, trace=False)
    return out


# revision 56
# speedup vs baseline: 1.1964x; 1.1964x over previous
"""Trainium2 Bass kernel for nn_Attention_81037442941065.

Dual-attention module (spatial [b,h,n,n] + channel [b,h,d,d]) with
B=2, N=2048, DIM=1024, 16 heads of d=64.

Sharding: 8 cores = (2 batches) x (4 head-groups of 4 heads).
Each core computes its batch/head-group slice end-to-end and produces a
partial (over head groups) output projection; the host sums the 4 group
partials per batch (the "all-reduce after to_out") and adds b_out.

Schedule (v2): the wall-clock pole is ScalarE's exp stream (128
ACTIVATEs of [128,1024], ~1.39us each, ~178us total).  Everything else
is arranged around keeping that stream gapless from as early as
possible:
  - only z1T/yhT (the S operands) are computed before the spatial loop;
  - xh, z2 + channel-attn logits, channel softmax, out2 and the final
    projection all run as an "aux" stream drained into the PE's idle
    slots inside the spatial loop (one matmul per drain slot, anchored
    to the S matmul of that slot so the scheduler cannot hoist them);
  - x is DMA'd in token-column blocks (host pre-blocks it) so each
    xh chunk only needs its own 256KB slice, letting AV consume
    xh_aug[j] within microseconds of spatial start;
  - output is written bf16 (host accumulates fp32) to halve the tail
    DMA; softmax denominators use reciprocal_approx_fast (~5x faster
    than InstReciprocal at ~18 correct bits).

Dtypes: all matmul operands bf16 (fp32 accumulation in PSUM); softmax
statistics fp32.  End-to-end relative error ~4e-3 vs fp32 reference.

Per-core layouts (everything "T" is [channels, tokens]):
  z1T, yhT   : 2 tiles [128, 2048]  (head h at rows 64*(h%2) of tile h//2)
  xh_aug     : 16 tiles [128, 260] (per 128-token chunk; per head 65
               cols = 64 channels + a ones column so the AV matmul also
               produces the softmax denominators)
  spatial    : S^T = yh @ z1^T computed [keys, queries]; the two heads
               of a pair run as concurrent PE row-tiles (base partition
               0/64); exp on ScalarE (scale 1/8 fused, no max
               subtraction - logits are small); AV matmul lhsT=[xh|1]
               accumulates over key chunks -> rows 0..63 =
               unnormalized out1^T, row 64 = sum of exp.
  channel    : logits accumulated per token-chunk into an SBUF fp32
               accumulator (PSUM stays free for the spatial loop);
               softmax via Exp+accum_out and per-partition reciprocal.
"""

import sys

for _p in ("/opt/trn_rl_repo", "/opt/pypackages"):
    if _p not in sys.path:
        sys.path.insert(0, _p)

import ml_dtypes
import numpy as np
from contextlib import ExitStack

import concourse.bacc as bacc
import concourse.mybir as mybir
import concourse.tile as tile
from concourse.tile import add_dep_helper
from concourse.bass_utils import run_bass_kernel_spmd

F32 = mybir.dt.float32
BF16 = mybir.dt.bfloat16
ATT = mybir.dt.bfloat16   # attention-internal matmul dtype
F8 = mybir.dt.float8e4    # e4m3: AV operands (P in (0,7.4], xh ~N(0,0.4))
DR = mybir.MatmulPerfMode.DoubleRow
EXP = mybir.ActivationFunctionType.Exp
COPY = mybir.ActivationFunctionType.Copy
XH8_H = 80                # fp8 xh head stride (16B-aligned for DoubleRow)
XH8_C = 4 * XH8_H         # fp8 xh chunk stride

B, N, DIM = 2, 2048, 1024
HEADS, DH = 16, 64
G = 4              # head groups == cores per batch
HG = HEADS // G    # heads per group (4)
CIN = HG * DH      # inner channels per core (256)
NCORES = 8
KC = DIM // 128    # contraction chunks for projections (8)
NCH = N // 128     # 128-token chunks (16)
SCALE = DH ** -0.5            # 1/8
CM_SCALE = SCALE / (N / DH)   # 1/256


def _build_program():
    nc = bacc.Bacc(
        "TRN2", target_bir_lowering=False, debug=False, num_devices=NCORES
    )

    # ---- DRAM I/O ----
    # xB is x^T re-blocked host-side: xB[i*128+p, k*128+j] = x^T[k*128+p,
    # i*128+j], so each token-chunk's projection operand is one contiguous
    # [128, 1024] DMA.
    xB_d = nc.dram_tensor("xB", [N, DIM], BF16, kind="ExternalInput").ap()
    yT_d = nc.dram_tensor("yT", [DIM, N], BF16, kind="ExternalInput").ap()
    zT_d = nc.dram_tensor("zT", [DIM, N], BF16, kind="ExternalInput").ap()
    # weights are host-blocked to [128, KC*CIN]: wB[p, k*CIN+o] = w[k*128+p, o]
    # so each weight matrix is a single contiguous DMA
    wsa1_d = nc.dram_tensor("w_sa1", [128, KC * CIN], BF16,
                            kind="ExternalInput").ap()
    wsa2_d = nc.dram_tensor("w_sa2", [128, KC * CIN], BF16,
                            kind="ExternalInput").ap()
    wse1_d = nc.dram_tensor("w_se1", [128, KC * CIN], BF16,
                            kind="ExternalInput").ap()
    wse2_d = nc.dram_tensor("w_se2", [128, KC * CIN], BF16,
                            kind="ExternalInput").ap()
    wout_d = nc.dram_tensor("w_out", [CIN, DIM], ATT, kind="ExternalInput").ap()
    outT_d = nc.dram_tensor("outT", [DIM, N], ATT, kind="ExternalOutput").ap()

    with tile.TileContext(nc) as tc, ExitStack() as ctx:
        ppool = ctx.enter_context(tc.tile_pool(name="persist", bufs=1))

        # Persistent tiles.
        z1T = [ppool.tile([128, N], ATT, tag=f"z1T{m}", name=f"z1T{m}")
               for m in range(2)]
        yhT = [ppool.tile([128, N], ATT, tag=f"yhT{m}", name=f"yhT{m}")
               for m in range(2)]
        xh_aug = [ppool.tile([128, HG * (DH + 1)], ATT, tag=f"xa{i}",
                             name=f"xa{i}") for i in range(NCH)]
        secm_sb = [ppool.tile([128, DH], ATT, tag=f"cm{p}", name=f"cm{p}")
                   for p in range(2)]
        rs = [ppool.tile([64, 1], F32, tag=f"rs{h}", name=f"rs{h}")
              for h in range(HG)]
        rcm = [ppool.tile([64, 1], F32, tag=f"rcm{h}", name=f"rcm{h}")
               for h in range(HG)]
        cmacc = ppool.tile([64, HG * DH], F32, tag="cmacc", name="cmacc")

        # inputs (weights as single blocked tiles)
        wsa1_t = ppool.tile([128, KC * CIN], BF16, tag="wsa1", name="wsa1")
        wsa2_t = ppool.tile([128, KC * CIN], BF16, tag="wsa2", name="wsa2")
        wse1_t = ppool.tile([128, KC * CIN], BF16, tag="wse1", name="wse1")
        wse2_t = ppool.tile([128, KC * CIN], BF16, tag="wse2", name="wse2")
        zTt = [ppool.tile([128, N], BF16, tag=f"z{k}", name=f"z{k}")
               for k in range(KC)]
        yTt = [ppool.tile([128, N], BF16, tag=f"y{k}", name=f"y{k}")
               for k in range(KC)]
        xcol = [ppool.tile([128, DIM], BF16, tag=f"xc{i}", name=f"xc{i}")
                for i in range(NCH)]
        wq = [ppool.tile([64, DIM], ATT, tag=f"wq{q}", name=f"wq{q}")
              for q in range(HG)]
        cat4 = [ppool.tile([64, N], ATT, tag=f"cat{h}", name=f"cat{h}")
                for h in range(HG)]

        ptpool = ctx.enter_context(tc.tile_pool(name="pt", bufs=4))
        tpool = ctx.enter_context(tc.tile_pool(name="tails", bufs=3))
        opool = ctx.enter_context(tc.tile_pool(name="oout", bufs=4))
        z2pool = ctx.enter_context(tc.tile_pool(name="z2s", bufs=3))

        # ---- All input DMAs on the sync queue in strict priority order:
        # wire order == need order (wsa/z/y gate the exp-stream start; wse/x
        # feed the pass-0 aux stream; wq is needed only from pass 4).
        # Scalar stays clean so z1T/yhT PSUM copies aren't queued behind
        # DGE ring waits.
        nc.sync.dma_start(wsa1_t[:], wsa1_d[:, :])
        nc.sync.dma_start(wsa2_t[:], wsa2_d[:, :])
        for k in range(KC):
            nc.sync.dma_start(zTt[k][:], zT_d[k * 128:(k + 1) * 128, :])
        for k in range(KC):
            nc.sync.dma_start(yTt[k][:], yT_d[k * 128:(k + 1) * 128, :])
        nc.sync.dma_start(wse1_t[:], wse1_d[:, :])
        nc.sync.dma_start(wse2_t[:], wse2_d[:, :])
        for i in range(NCH):
            nc.sync.dma_start(xcol[i][:], xB_d[i * 128:(i + 1) * 128, :])
        for q in range(HG):
            nc.sync.dma_start(wq[q][:], wout_d[q * 64:(q + 1) * 64, :])

        # constants on gpsimd (ones columns needed by the first AV)
        nc.gpsimd.memset(cmacc[:], 0.0)
        for i in range(NCH):
            dst = xh_aug[i][:].rearrange("p (h c) -> p h c", c=DH + 1)
            nc.gpsimd.memset(dst[:, :, DH:DH + 1], 1.0)

        # PE warmup: HAM boots at 1.2 GHz and needs ~3.4us of sustained
        # matmul activity to unthrottle.  Burn that in on a zeroed tile
        # before the first real operand lands so the projections run at
        # 2.4 GHz from the start.
        warm0 = ppool.tile([128, 512], BF16, tag="warm0", name="warm0")
        nc.vector.memset(warm0[:], 0.0)

        # cat4 accumulates out1 (tails) and out2 (aux adds) in either order
        for h in range(HG):
            nc.vector.memset(cat4[h][:], 0.0)

        # ============ Pre-spatial: z1T / yhT projections only ============
        # k-major emission across all 8 (m, nb) chains: each arriving
        # zTt[k]/yTt[k] DMA tile unlocks 8 consecutive matmuls, so the PE
        # tracks the DMA feed rate instead of head-of-line-blocking on one
        # chain's next k-tile.  bufs=8 = the whole PSUM (spatial pools open
        # after this scope closes).
        with tc.tile_pool(name="psp", bufs=8, space="PSUM") as psp:
            pw = psp.tile([128, 512], F32, tag="pj", name="pwarm")
            for w in range(20):
                nc.tensor.matmul(pw[:], lhsT=warm0[:, 0:128], rhs=warm0[:],
                                 start=(w == 0), stop=(w == 19))
            for rnd, (dst, wt, srcs) in enumerate(
                    ((z1T, wsa1_t, zTt), (yhT, wsa2_t, yTt))):
                pss = {}
                for m in range(2):
                    for nb in range(4):
                        pss[(m, nb)] = psp.tile([128, 512], F32, tag="pj",
                                                name=f"ps{rnd}{m}{nb}")
                for k in range(KC):
                    for m in range(2):
                        for nb in range(4):
                            nc.tensor.matmul(
                                pss[(m, nb)][:],
                                lhsT=wt[:, k * CIN + m * 128:
                                        k * CIN + (m + 1) * 128],
                                rhs=srcs[k][:, nb * 512:(nb + 1) * 512],
                                start=(k == 0), stop=(k == KC - 1),
                            )
                # m=0 copies first (the first spatial pass reads only m=0),
                # split across scalar and vector so the copy tail halves;
                # all m=1 copies go to vector so they can't delay the first
                # exps behind them in the scalar queue
                for m in range(2):
                    for nb in range(4):
                        src_ps = pss[(m, nb)][:]
                        dslice = dst[m][:, nb * 512:(nb + 1) * 512]
                        if m == 0 and nb % 2 == 0:
                            nc.scalar.copy(dslice, src_ps)
                        else:
                            nc.vector.tensor_copy(dslice, src_ps)

        # ============ Spatial loop with full aux stream ============
        # PSUM: S 2x[128,1024] (4 banks) + av 2x[128,512] (2 banks) +
        # aux 2x[128,512] (2 banks) = 8 banks exactly.
        with tc.tile_pool(name="psS", bufs=2, space="PSUM") as psS, \
             tc.tile_pool(name="psAV", bufs=2, space="PSUM") as psAV, \
             tc.tile_pool(name="psaux", bufs=2, space="PSUM") as psaux:

            # Aux matmul stream: xh / z2+channel-logits / out2 / final
            # projection, one PE instruction per thunk, drained inside the
            # spatial j-loops so the PE always has ready work while ScalarE
            # runs the exps.
            aux_thunks = []
            final_psf = {}
            xh_ps = {}
            z2_ps = {}
            cm_ps = {}
            z2n_t = {}

            def emit_xh_mm(i, k):
                if k == 0:
                    xh_ps[i] = psaux.tile([128, 512], F32, tag="aux",
                                          name=f"psx{i}")
                ps = xh_ps[i]
                mm = nc.tensor.matmul(
                    ps[:, 0:CIN],
                    lhsT=xcol[i][:, k * 128:(k + 1) * 128],
                    rhs=wse1_t[:, k * CIN:(k + 1) * CIN],
                    start=(k == 0), stop=(k == KC - 1),
                )
                if k == KC - 1:
                    src = ps[:, 0:CIN].rearrange("p (h c) -> p h c", c=DH)
                    dst = xh_aug[i][:].rearrange("p (h c) -> p h c", c=DH + 1)
                    nc.vector.tensor_copy(dst[:, :, 0:DH], src)
                    del xh_ps[i]
                return mm

            def emit_z2_mm(i, k):
                if k == 0:
                    z2_ps[i] = psaux.tile([128, 512], F32, tag="aux",
                                          name=f"psz2_{i}")
                ps = z2_ps[i]
                mm = nc.tensor.matmul(
                    ps[:, 0:CIN],
                    lhsT=zTt[k][:, i * 128:(i + 1) * 128],
                    rhs=wse2_t[:, k * CIN:(k + 1) * CIN],
                    start=(k == 0), stop=(k == KC - 1),
                )
                if k == KC - 1:
                    z2n = z2pool.tile([128, CIN], ATT, tag="z2n",
                                      name=f"z2n{i}")
                    nc.vector.tensor_copy(z2n[:], ps[:, 0:CIN])
                    z2n_t[i] = z2n
                    del z2_ps[i]
                return mm

            def emit_cm_mm(i, h):
                if h == 0:
                    cm_ps[i] = psaux.tile([128, 512], F32, tag="aux",
                                          name=f"pscm{i}")
                ps = cm_ps[i]
                mm = nc.tensor.matmul(
                    ps[0:64, h * DH:(h + 1) * DH],
                    lhsT=xh_aug[i][:, 65 * h:65 * h + DH],
                    rhs=z2n_t[i][:, DH * h:DH * (h + 1)],
                    start=True, stop=True,
                )
                if h == HG - 1:
                    nc.vector.tensor_add(cmacc[:], ps[0:64, 0:HG * DH],
                                         cmacc[:])
                    del cm_ps[i]
                    del z2n_t[i]
                    if i == NCH - 1:
                        # channel-attn softmax, DMA'd into pair-packed secm_sb
                        for hh in range(HG):
                            p_, off = hh // 2, 64 * (hh % 2)
                            st = z2pool.tile([64, DH], ATT, tag="cmstage",
                                             name=f"cmstage{hh}")
                            nc.scalar.activation(
                                st[:], cmacc[:, hh * DH:(hh + 1) * DH], EXP,
                                scale=CM_SCALE, accum_out=rs[hh][0:64, 0:1])
                            nc.vector.reciprocal(rcm[hh][0:64, 0:1],
                                                 rs[hh][0:64, 0:1])
                            nc.vector.tensor_scalar_mul(st[:], st[:],
                                                        rcm[hh][0:64, 0:1])
                            nc.sync.dma_start(secm_sb[p_][off:off + 64, :],
                                              st[:])
                return mm

            def emit_out2(h, nb):
                p_, off = h // 2, 64 * (h % 2)
                pso = psaux.tile([128, 512], F32, tag="aux",
                                 name=f"pso{h}{nb}")
                mm = nc.tensor.matmul(
                    pso[0:64, :],
                    lhsT=secm_sb[p_][off:off + 64, :],
                    rhs=yhT[p_][off:off + 64, nb * 512:(nb + 1) * 512],
                    start=True, stop=True,
                )
                dst = cat4[h][:, nb * 512:(nb + 1) * 512]
                nc.vector.tensor_add(dst, pso[0:64, :], dst)
                return mm

            def emit_final_mm(d, nb, q):
                if q == 0:
                    final_psf[(d, nb)] = psaux.tile(
                        [128, 512], F32, tag="aux", name=f"psf{d}{nb}")[:]
                psf = final_psf[(d, nb)]
                mm = nc.tensor.matmul(
                    psf,
                    lhsT=wq[q][:, d * 128:(d + 1) * 128],
                    rhs=cat4[q][:, nb * 512:(nb + 1) * 512],
                    start=(q == 0), stop=(q == HG - 1),
                )
                if q == HG - 1:
                    ob = opool.tile([128, 512], ATT, tag="ob",
                                    name=f"ob{d}{nb}")
                    if nb == 3:
                        nc.scalar.copy(ob[:], psf)
                    else:
                        nc.vector.tensor_copy(ob[:], psf)
                    nc.sync.dma_start(
                        outT_d[d * 128:(d + 1) * 128,
                               nb * 512:(nb + 1) * 512],
                        ob[:],
                    )
                return mm

            # static aux queue: all xh chunks, then z2+cm per chunk, then
            # out2; finals are appended as their cat4 blocks complete
            for i in range(NCH):
                for k in range(KC):
                    aux_thunks.append(lambda i=i, k=k: emit_xh_mm(i, k))
            for i in range(NCH):
                for k in range(KC):
                    aux_thunks.append(lambda i=i, k=k: emit_z2_mm(i, k))
                for h in range(HG):
                    aux_thunks.append(lambda i=i, h=h: emit_cm_mm(i, h))
            for h in range(HG):
                for nb in range(4):
                    aux_thunks.append(lambda h=h, nb=nb: emit_out2(h, nb))

            def queue_finals(nb, ds=range(8)):
                for d in ds:
                    for q in range(HG):
                        aux_thunks.append(
                            lambda d=d, nb=nb, q=q: emit_final_mm(d, nb, q))

            def drain_aux(k, anchor=None):
                # anchor pins the aux matmul into this drain slot's position
                # in the PE stream - the scheduler's gap-filler otherwise
                # hoists thunks into earlier windows where their inputs are
                # still several microseconds from ready
                for _ in range(k):
                    if aux_thunks:
                        mm = aux_thunks.pop(0)()
                        if anchor is not None and mm is not None:
                            add_dep_helper(mm.ins, anchor.ins, sync=False,
                                           reason="pin aux to drain slot")

            # drains per j-slot for each pass (pass = 2*ib + p_): front-load
            # xh (consumed by AV from pass 0) and z2/cm, then pace the
            # remaining 128 aux matmuls so no pass runs dry (HAM re-throttles
            # the PE clock if it idles)
            DRAIN_SCHED = [8, 4, 4, 4, 2, 2, 2, 2]

            def make_tail(p_, ib, avs, ptt_last):
                # Two-part tail.  Head (next iteration, j==0): the last
                # j-pair's AV matmuls, the avsb copies that release the AV
                # PSUM banks, and a small DMA that spreads each denominator
                # row [1,512] to [64,8] so its reciprocal is ~150ns on DVE
                # instead of a 3.3us FIFO-hogging [1,512] InstReciprocal.
                # Norm (j==2): reciprocal, DMA back, broadcast, scale, add.
                icol = ib * 512
                avsbs, d64s = [], []

                def emit_head():
                    for hh in range(2):
                        h = 2 * p_ + hh
                        nc.tensor.matmul(
                            avs[hh][0:DH + 1, :],
                            lhsT=xh_aug[NCH - 1][:, 65 * h:65 * h + DH + 1],
                            rhs=ptt_last[:, 512 * hh:512 * hh + 512],
                            start=False, stop=True,
                        )
                    for hh in range(2):
                        avsb = tpool.tile([DH + 1, 512], F32, tag="avsb",
                                          name=f"avsb{p_}{ib}{hh}")
                        nc.vector.tensor_copy(avsb[:], avs[hh][0:DH + 1, :])
                        avsbs.append(avsb)
                        if ib != 3:
                            d64 = tpool.tile([64, 8], F32, tag="d64",
                                             name=f"d64_{p_}{ib}{hh}")
                            nc.sync.dma_start(d64[:], avsb[DH:DH + 1, :])
                            d64s.append(d64)

                def emit_norm():
                    for hh in range(2):
                        h = 2 * p_ + hh
                        rc = tpool.tile([1, 512], F32, tag="rc",
                                        name=f"rc{p_}{ib}{hh}")
                        if ib == 3:
                            # latency-critical last tails: direct reciprocal
                            # beats the two-DMA-hop partition-spread version
                            nc.vector.reciprocal(rc[:],
                                                 avsbs[hh][DH:DH + 1, :])
                        else:
                            d64r = tpool.tile([64, 8], F32, tag="d64r",
                                              name=f"d64r{p_}{ib}{hh}")
                            nc.vector.reciprocal(d64r[:], d64s[hh][:])
                            nc.sync.dma_start(rc[:], d64r[:])
                        bc = tpool.tile([64, 512], F32, tag="bc",
                                        name=f"bc{p_}{ib}{hh}")
                        nc.gpsimd.partition_broadcast(bc[:], rc[:])
                        tmp = tpool.tile([64, 512], F32, tag="tmp",
                                         name=f"tmp{p_}{ib}{hh}")
                        nc.vector.tensor_mul(tmp[:], avsbs[hh][0:DH, :], bc[:])
                        dst = cat4[h][:, icol:icol + 512]
                        nc.vector.tensor_add(dst, tmp[:], dst)
                return emit_head, emit_norm

            pending_tail = None
            # --- spatial attention: iterations (ib 512-block, pair),
            #     processing key chunks two at a time (j-pairs) ---
            for ib in range(4):
                for p_ in range(2):
                    # nb's cat4 block is complete once BOTH pairs' tails ran;
                    # the second pair's tails execute during (ib+1, p0), so
                    # finals(nb) join the aux queue at (ib+1, p1)
                    if p_ == 1 and ib >= 1:
                        queue_finals(ib - 1)
                    icol = ib * 512
                    ndrain = DRAIN_SCHED[2 * ib + p_]
                    avs = [psAV.tile([128, 512], F32, tag="av",
                                     name=f"av{p_}{ib}{q}") for q in range(2)]
                    pair_t = [None] * NCH
                    for j in range(NCH):  # key chunks
                        spt = psS.tile([128, 1024], F32, tag="S",
                                       name=f"S{p_}{ib}{j}")
                        s_anchor = None
                        for hh in range(2):
                            off = 64 * hh
                            s_anchor = nc.tensor.matmul(
                                spt[:, 512 * hh:512 * hh + 512],
                                lhsT=yhT[p_][off:off + 64,
                                             j * 128:(j + 1) * 128],
                                rhs=z1T[p_][off:off + 64, icol:icol + 512],
                                start=True, stop=True,
                            )
                        ptt = ptpool.tile([128, 1024], ATT, tag="pt",
                                          name=f"pt{p_}{ib}{j}")
                        nc.scalar.activation(ptt[:], spt[:], EXP, scale=SCALE)
                        pair_t[j] = ptt
                        if pending_tail is not None:
                            if j == 0:
                                pending_tail[0]()
                            elif j == 2:
                                pending_tail[1]()
                                pending_tail = None
                        drain_aux(ndrain, s_anchor)
                        if j > 0:
                            for hh in range(2):
                                h = 2 * p_ + hh
                                nc.tensor.matmul(
                                    avs[hh][0:DH + 1, :],
                                    lhsT=xh_aug[j - 1][:, 65 * h:65 * h + DH + 1],
                                    rhs=pair_t[j - 1][:, 512 * hh:512 * hh + 512],
                                    start=(j == 1), stop=False,
                                )
                    pending_tail = make_tail(p_, ib, avs, pair_t[NCH - 1])
            pending_tail[0]()
            pending_tail[1]()
            # warm-keeper: the last tail's normalization chain
            # (dma->recip->dma->broadcast->mul->add) leaves the PE idle just
            # long enough for HAM to re-throttle the clock to 1.2 GHz right
            # before the last 32 final-projection matmuls.  Keep it busy
            # with throwaway matmuls whose results are never read.
            warm = psaux.tile([128, 512], F32, tag="aux", name="warmk")
            for w in range(40):
                nc.tensor.matmul(
                    warm[:],
                    lhsT=yhT[0][0:128, 0:128],
                    rhs=z1T[0][0:128, 0:512],
                    start=(w == 0), stop=(w == 39),
                )
            queue_finals(3)
            drain_aux(len(aux_thunks))

    nc.compile()
    return nc


_NC_CACHE = {}


def _get_program():
    if "nc" not in _NC_CACHE:
        _NC_CACHE["nc"] = _build_program()
    return _NC_CACHE["nc"]


def _prep_input_maps(x, y, z, w_sa1, w_sa2, w_se1, w_se2, w_out):
    bf16 = lambda a: np.ascontiguousarray(
        np.asarray(a, dtype=np.float32).astype(ml_dtypes.bfloat16))
    # wB[p, k*CIN+o] = w[k*128+p, o]
    wblk = lambda w: w.reshape(KC, 128, CIN).transpose(1, 0, 2) \
                      .reshape(128, KC * CIN)
    maps = []
    for c in range(NCORES):
        b, g = divmod(c, G)
        sl = slice(g * CIN, (g + 1) * CIN)
        xT = np.asarray(x)[b].T  # [DIM, N]
        # xB[i*128+p, k*128+j] = xT[k*128+p, i*128+j]
        xBlk = xT.reshape(KC, 128, NCH, 128).transpose(2, 1, 0, 3) \
                 .reshape(N, DIM)
        maps.append({
            "xB": bf16(xBlk),
            "yT": bf16(np.asarray(y)[b].T),
            "zT": bf16(np.asarray(z)[b].T),
            "w_sa1": bf16(wblk(np.asarray(w_sa1)[:, sl])),
            "w_sa2": bf16(wblk(np.asarray(w_sa2)[:, sl])),
            "w_se1": bf16(wblk(np.asarray(w_se1)[:, sl])),
            "w_se2": bf16(wblk(np.asarray(w_se2)[:, sl])),
            "w_out": bf16(np.asarray(w_out)[sl, :]),
        })
    return maps


def run(inputs, trace=False, trace_kwargs=None):
    """Run on hardware; returns (full_output, BassKernelResults)."""
    nc = _get_program()
    in_maps = _prep_input_maps(
        inputs["x"], inputs["y"], inputs["z"],
        inputs["w_sa1"], inputs["w_sa2"], inputs["w_se1"], inputs["w_se2"],
        inputs["w_out"],
    )
    res = run_bass_kernel_spmd(
        nc, in_maps, list(range(NCORES)), trace=trace,
        trace_kwargs=trace_kwargs or {},
    )
    out = np.zeros((B, N, DIM), dtype=np.float32)
    for c in range(NCORES):
        b, _g = divmod(c, G)
        out[b] += np.asarray(res.results[c]["outT"]).astype(np.float32).T
    out += np.asarray(inputs["b_out"], dtype=np.float32)
    return out, res


def kernel(**inputs) -> np.ndarray:
    out, _ = run(inputs, trace=False)
    return out


# revision 57
# speedup vs baseline: 1.2062x; 1.0081x over previous
"""Trainium2 Bass kernel for nn_Attention_81037442941065.

Dual-attention module (spatial [b,h,n,n] + channel [b,h,d,d]) with
B=2, N=2048, DIM=1024, 16 heads of d=64.

Sharding: 8 cores = (2 batches) x (4 head-groups of 4 heads).
Each core computes its batch/head-group slice end-to-end and produces a
partial (over head groups) output projection; the host sums the 4 group
partials per batch (the "all-reduce after to_out") and adds b_out.

Schedule (v2): the wall-clock pole is ScalarE's exp stream (128
ACTIVATEs of [128,1024], ~1.39us each, ~178us total).  Everything else
is arranged around keeping that stream gapless from as early as
possible:
  - only z1T/yhT (the S operands) are computed before the spatial loop;
  - xh, z2 + channel-attn logits, channel softmax, out2 and the final
    projection all run as an "aux" stream drained into the PE's idle
    slots inside the spatial loop (one matmul per drain slot, anchored
    to the S matmul of that slot so the scheduler cannot hoist them);
  - x is DMA'd in token-column blocks (host pre-blocks it) so each
    xh chunk only needs its own 256KB slice, letting AV consume
    xh_aug[j] within microseconds of spatial start;
  - input DMAs are a single priority-ordered sync-queue stream (wire
    order == need order) and the PE is pre-warmed with throwaway
    matmuls so the HAM clock gate is at 2.4 GHz before real work;
  - spatial-tail softmax denominators are DMA-spread [1,512]->[64,8]
    so their reciprocal costs ~150ns instead of a 3.3us FIFO-hogging
    InstReciprocal (the latency-critical last tails use the direct
    form); output is written bf16 (host accumulates fp32).

Dtypes: all matmul operands bf16 (fp32 accumulation in PSUM); softmax
statistics fp32.  End-to-end relative error ~4e-3 vs fp32 reference.

Per-core layouts (everything "T" is [channels, tokens]):
  z1T, yhT   : 2 tiles [128, 2048]  (head h at rows 64*(h%2) of tile h//2)
  xh_aug     : 16 tiles [128, 260] (per 128-token chunk; per head 65
               cols = 64 channels + a ones column so the AV matmul also
               produces the softmax denominators)
  spatial    : S^T = yh @ z1^T computed [keys, queries]; the two heads
               of a pair run as concurrent PE row-tiles (base partition
               0/64); exp on ScalarE (scale 1/8 fused, no max
               subtraction - logits are small); AV matmul lhsT=[xh|1]
               accumulates over key chunks -> rows 0..63 =
               unnormalized out1^T, row 64 = sum of exp.
  channel    : logits accumulated per token-chunk into an SBUF fp32
               accumulator (PSUM stays free for the spatial loop);
               softmax via Exp+accum_out and per-partition reciprocal.
"""

import sys

for _p in ("/opt/trn_rl_repo", "/opt/pypackages"):
    if _p not in sys.path:
        sys.path.insert(0, _p)

import ml_dtypes
import numpy as np
from contextlib import ExitStack

import concourse.bacc as bacc
import concourse.mybir as mybir
import concourse.tile as tile
from concourse.tile import add_dep_helper
from concourse.bass_utils import run_bass_kernel_spmd

F32 = mybir.dt.float32
BF16 = mybir.dt.bfloat16
ATT = mybir.dt.bfloat16   # attention-internal matmul dtype
F8 = mybir.dt.float8e4    # e4m3: AV operands (P in (0,7.4], xh ~N(0,0.4))
DR = mybir.MatmulPerfMode.DoubleRow
EXP = mybir.ActivationFunctionType.Exp
COPY = mybir.ActivationFunctionType.Copy
XH8_H = 80                # fp8 xh head stride (16B-aligned for DoubleRow)
XH8_C = 4 * XH8_H         # fp8 xh chunk stride

B, N, DIM = 2, 2048, 1024
HEADS, DH = 16, 64
G = 4              # head groups == cores per batch
HG = HEADS // G    # heads per group (4)
CIN = HG * DH      # inner channels per core (256)
NCORES = 8
KC = DIM // 128    # contraction chunks for projections (8)
NCH = N // 128     # 128-token chunks (16)
SCALE = DH ** -0.5            # 1/8
CM_SCALE = SCALE / (N / DH)   # 1/256


def _build_program():
    nc = bacc.Bacc(
        "TRN2", target_bir_lowering=False, debug=False, num_devices=NCORES
    )

    # ---- DRAM I/O ----
    # xB is x^T re-blocked host-side: xB[i*128+p, k*128+j] = x^T[k*128+p,
    # i*128+j], so each token-chunk's projection operand is one contiguous
    # [128, 1024] DMA.
    xB_d = nc.dram_tensor("xB", [N, DIM], BF16, kind="ExternalInput").ap()
    yT_d = nc.dram_tensor("yT", [DIM, N], BF16, kind="ExternalInput").ap()
    zT_d = nc.dram_tensor("zT", [DIM, N], BF16, kind="ExternalInput").ap()
    # weights are host-blocked to [128, KC*CIN]: wB[p, k*CIN+o] = w[k*128+p, o]
    # so each weight matrix is a single contiguous DMA
    wsa1_d = nc.dram_tensor("w_sa1", [128, KC * CIN], BF16,
                            kind="ExternalInput").ap()
    wsa2_d = nc.dram_tensor("w_sa2", [128, KC * CIN], BF16,
                            kind="ExternalInput").ap()
    wse1_d = nc.dram_tensor("w_se1", [128, KC * CIN], BF16,
                            kind="ExternalInput").ap()
    wse2_d = nc.dram_tensor("w_se2", [128, KC * CIN], BF16,
                            kind="ExternalInput").ap()
    wout_d = nc.dram_tensor("w_out", [CIN, DIM], ATT, kind="ExternalInput").ap()
    outT_d = nc.dram_tensor("outT", [DIM, N], ATT, kind="ExternalOutput").ap()

    with tile.TileContext(nc) as tc, ExitStack() as ctx:
        ppool = ctx.enter_context(tc.tile_pool(name="persist", bufs=1))

        # Persistent tiles.
        z1T = [ppool.tile([128, N], ATT, tag=f"z1T{m}", name=f"z1T{m}")
               for m in range(2)]
        yhT = [ppool.tile([128, N], ATT, tag=f"yhT{m}", name=f"yhT{m}")
               for m in range(2)]
        xh_aug = [ppool.tile([128, HG * (DH + 1)], ATT, tag=f"xa{i}",
                             name=f"xa{i}") for i in range(NCH)]
        secm_sb = [ppool.tile([128, DH], ATT, tag=f"cm{p}", name=f"cm{p}")
                   for p in range(2)]
        rs = [ppool.tile([64, 1], F32, tag=f"rs{h}", name=f"rs{h}")
              for h in range(HG)]
        rcm = [ppool.tile([64, 1], F32, tag=f"rcm{h}", name=f"rcm{h}")
               for h in range(HG)]
        cmacc = ppool.tile([64, HG * DH], F32, tag="cmacc", name="cmacc")

        # inputs (weights as single blocked tiles)
        wsa1_t = ppool.tile([128, KC * CIN], BF16, tag="wsa1", name="wsa1")
        wsa2_t = ppool.tile([128, KC * CIN], BF16, tag="wsa2", name="wsa2")
        wse1_t = ppool.tile([128, KC * CIN], BF16, tag="wse1", name="wse1")
        wse2_t = ppool.tile([128, KC * CIN], BF16, tag="wse2", name="wse2")
        zTt = [ppool.tile([128, N], BF16, tag=f"z{k}", name=f"z{k}")
               for k in range(KC)]
        yTt = [ppool.tile([128, N], BF16, tag=f"y{k}", name=f"y{k}")
               for k in range(KC)]
        xcol = [ppool.tile([128, DIM], BF16, tag=f"xc{i}", name=f"xc{i}")
                for i in range(NCH)]
        wq = [ppool.tile([64, DIM], ATT, tag=f"wq{q}", name=f"wq{q}")
              for q in range(HG)]
        cat4 = [ppool.tile([64, N], ATT, tag=f"cat{h}", name=f"cat{h}")
                for h in range(HG)]

        ptpool = ctx.enter_context(tc.tile_pool(name="pt", bufs=4))
        tpool = ctx.enter_context(tc.tile_pool(name="tails", bufs=3))
        opool = ctx.enter_context(tc.tile_pool(name="oout", bufs=4))
        z2pool = ctx.enter_context(tc.tile_pool(name="z2s", bufs=3))

        # ---- All input DMAs on the sync queue in strict priority order:
        # wire order == need order (wsa/z/y gate the exp-stream start; wse/x
        # feed the pass-0 aux stream; wq is needed only from pass 4).
        # Scalar stays clean so z1T/yhT PSUM copies aren't queued behind
        # DGE ring waits.
        nc.sync.dma_start(wsa1_t[:], wsa1_d[:, :])
        nc.sync.dma_start(wsa2_t[:], wsa2_d[:, :])
        for k in range(KC):
            nc.sync.dma_start(zTt[k][:], zT_d[k * 128:(k + 1) * 128, :])
        for k in range(KC):
            nc.sync.dma_start(yTt[k][:], yT_d[k * 128:(k + 1) * 128, :])
        nc.sync.dma_start(wse1_t[:], wse1_d[:, :])
        nc.sync.dma_start(wse2_t[:], wse2_d[:, :])
        for i in range(NCH):
            nc.sync.dma_start(xcol[i][:], xB_d[i * 128:(i + 1) * 128, :])
        for q in range(HG):
            nc.sync.dma_start(wq[q][:], wout_d[q * 64:(q + 1) * 64, :])

        # constants on gpsimd (ones columns needed by the first AV)
        nc.gpsimd.memset(cmacc[:], 0.0)
        for i in range(NCH):
            dst = xh_aug[i][:].rearrange("p (h c) -> p h c", c=DH + 1)
            nc.gpsimd.memset(dst[:, :, DH:DH + 1], 1.0)

        # PE warmup: HAM boots at 1.2 GHz and needs ~3.4us of sustained
        # matmul activity to unthrottle.  Burn that in on a zeroed tile
        # before the first real operand lands so the projections run at
        # 2.4 GHz from the start.
        warm0 = ppool.tile([128, 512], BF16, tag="warm0", name="warm0")
        nc.vector.memset(warm0[:], 0.0)

        # cat4 accumulates out1 (tails) and out2 (aux adds) in either order
        for h in range(HG):
            nc.vector.memset(cat4[h][:], 0.0)

        # ============ Pre-spatial: z1T / yhT projections only ============
        # k-major emission across all 8 (m, nb) chains: each arriving
        # zTt[k]/yTt[k] DMA tile unlocks 8 consecutive matmuls, so the PE
        # tracks the DMA feed rate instead of head-of-line-blocking on one
        # chain's next k-tile.  bufs=8 = the whole PSUM (spatial pools open
        # after this scope closes).
        with tc.tile_pool(name="psp", bufs=8, space="PSUM") as psp:
            pw = psp.tile([128, 512], F32, tag="pj", name="pwarm")
            for w in range(20):
                nc.tensor.matmul(pw[:], lhsT=warm0[:, 0:128], rhs=warm0[:],
                                 start=(w == 0), stop=(w == 19))
            for rnd, (dst, wt, srcs) in enumerate(
                    ((z1T, wsa1_t, zTt), (yhT, wsa2_t, yTt))):
                pss = {}
                for m in range(2):
                    for nb in range(4):
                        pss[(m, nb)] = psp.tile([128, 512], F32, tag="pj",
                                                name=f"ps{rnd}{m}{nb}")
                for k in range(KC):
                    for m in range(2):
                        for nb in range(4):
                            nc.tensor.matmul(
                                pss[(m, nb)][:],
                                lhsT=wt[:, k * CIN + m * 128:
                                        k * CIN + (m + 1) * 128],
                                rhs=srcs[k][:, nb * 512:(nb + 1) * 512],
                                start=(k == 0), stop=(k == KC - 1),
                            )
                # m=0 copies first (the first spatial pass reads only m=0),
                # split across scalar and vector so the copy tail halves;
                # all m=1 copies go to vector so they can't delay the first
                # exps behind them in the scalar queue
                for m in range(2):
                    for nb in range(4):
                        src_ps = pss[(m, nb)][:]
                        dslice = dst[m][:, nb * 512:(nb + 1) * 512]
                        if m == 0 and nb % 2 == 0:
                            nc.scalar.copy(dslice, src_ps)
                        else:
                            nc.vector.tensor_copy(dslice, src_ps)

        # ============ Spatial loop with full aux stream ============
        # PSUM: S 2x[128,1024] (4 banks) + av 2x[128,512] (2 banks) +
        # aux 2x[128,512] (2 banks) = 8 banks exactly.
        with tc.tile_pool(name="psS", bufs=2, space="PSUM") as psS, \
             tc.tile_pool(name="psAV", bufs=2, space="PSUM") as psAV, \
             tc.tile_pool(name="psaux", bufs=2, space="PSUM") as psaux:

            # Aux matmul stream: xh / z2+channel-logits / out2 / final
            # projection, one PE instruction per thunk, drained inside the
            # spatial j-loops so the PE always has ready work while ScalarE
            # runs the exps.
            aux_thunks = []
            final_psf = {}
            xh_ps = {}
            z2_ps = {}
            cm_ps = {}
            z2n_t = {}

            def emit_xh_mm(i, k):
                if k == 0:
                    xh_ps[i] = psaux.tile([128, 512], F32, tag="aux",
                                          name=f"psx{i}")
                ps = xh_ps[i]
                mm = nc.tensor.matmul(
                    ps[:, 0:CIN],
                    lhsT=xcol[i][:, k * 128:(k + 1) * 128],
                    rhs=wse1_t[:, k * CIN:(k + 1) * CIN],
                    start=(k == 0), stop=(k == KC - 1),
                )
                if k == KC - 1:
                    src = ps[:, 0:CIN].rearrange("p (h c) -> p h c", c=DH)
                    dst = xh_aug[i][:].rearrange("p (h c) -> p h c", c=DH + 1)
                    nc.vector.tensor_copy(dst[:, :, 0:DH], src)
                    del xh_ps[i]
                return mm

            def emit_z2_mm(i, k):
                if k == 0:
                    z2_ps[i] = psaux.tile([128, 512], F32, tag="aux",
                                          name=f"psz2_{i}")
                ps = z2_ps[i]
                mm = nc.tensor.matmul(
                    ps[:, 0:CIN],
                    lhsT=zTt[k][:, i * 128:(i + 1) * 128],
                    rhs=wse2_t[:, k * CIN:(k + 1) * CIN],
                    start=(k == 0), stop=(k == KC - 1),
                )
                if k == KC - 1:
                    z2n = z2pool.tile([128, CIN], ATT, tag="z2n",
                                      name=f"z2n{i}")
                    nc.vector.tensor_copy(z2n[:], ps[:, 0:CIN])
                    z2n_t[i] = z2n
                    del z2_ps[i]
                return mm

            def emit_cm_mm(i, h):
                if h == 0:
                    cm_ps[i] = psaux.tile([128, 512], F32, tag="aux",
                                          name=f"pscm{i}")
                ps = cm_ps[i]
                mm = nc.tensor.matmul(
                    ps[0:64, h * DH:(h + 1) * DH],
                    lhsT=xh_aug[i][:, 65 * h:65 * h + DH],
                    rhs=z2n_t[i][:, DH * h:DH * (h + 1)],
                    start=True, stop=True,
                )
                if h == HG - 1:
                    nc.vector.tensor_add(cmacc[:], ps[0:64, 0:HG * DH],
                                         cmacc[:])
                    del cm_ps[i]
                    del z2n_t[i]
                    if i == NCH - 1:
                        # channel-attn softmax, DMA'd into pair-packed secm_sb
                        for hh in range(HG):
                            p_, off = hh // 2, 64 * (hh % 2)
                            st = z2pool.tile([64, DH], ATT, tag="cmstage",
                                             name=f"cmstage{hh}")
                            nc.scalar.activation(
                                st[:], cmacc[:, hh * DH:(hh + 1) * DH], EXP,
                                scale=CM_SCALE, accum_out=rs[hh][0:64, 0:1])
                            nc.vector.reciprocal(rcm[hh][0:64, 0:1],
                                                 rs[hh][0:64, 0:1])
                            nc.vector.tensor_scalar_mul(st[:], st[:],
                                                        rcm[hh][0:64, 0:1])
                            nc.sync.dma_start(secm_sb[p_][off:off + 64, :],
                                              st[:])
                return mm

            def emit_out2(h, nb):
                p_, off = h // 2, 64 * (h % 2)
                pso = psaux.tile([128, 512], F32, tag="aux",
                                 name=f"pso{h}{nb}")
                mm = nc.tensor.matmul(
                    pso[0:64, :],
                    lhsT=secm_sb[p_][off:off + 64, :],
                    rhs=yhT[p_][off:off + 64, nb * 512:(nb + 1) * 512],
                    start=True, stop=True,
                )
                dst = cat4[h][:, nb * 512:(nb + 1) * 512]
                nc.vector.tensor_add(dst, pso[0:64, :], dst)
                return mm

            def emit_final_mm(d, nb, q):
                if q == 0:
                    final_psf[(d, nb)] = psaux.tile(
                        [128, 512], F32, tag="aux", name=f"psf{d}{nb}")[:]
                psf = final_psf[(d, nb)]
                mm = nc.tensor.matmul(
                    psf,
                    lhsT=wq[q][:, d * 128:(d + 1) * 128],
                    rhs=cat4[q][:, nb * 512:(nb + 1) * 512],
                    start=(q == 0), stop=(q == HG - 1),
                )
                if q == HG - 1:
                    ob = opool.tile([128, 512], ATT, tag="ob",
                                    name=f"ob{d}{nb}")
                    if nb == 3:
                        nc.scalar.copy(ob[:], psf)
                    else:
                        nc.vector.tensor_copy(ob[:], psf)
                    nc.sync.dma_start(
                        outT_d[d * 128:(d + 1) * 128,
                               nb * 512:(nb + 1) * 512],
                        ob[:],
                    )
                return mm

            # static aux queue: all xh chunks, then z2+cm per chunk, then
            # out2; finals are appended as their cat4 blocks complete
            for i in range(NCH):
                for k in range(KC):
                    aux_thunks.append(lambda i=i, k=k: emit_xh_mm(i, k))
            for i in range(NCH):
                for k in range(KC):
                    aux_thunks.append(lambda i=i, k=k: emit_z2_mm(i, k))
                for h in range(HG):
                    aux_thunks.append(lambda i=i, h=h: emit_cm_mm(i, h))
            for h in range(HG):
                for nb in range(4):
                    aux_thunks.append(lambda h=h, nb=nb: emit_out2(h, nb))

            def queue_finals(nb, ds=range(8)):
                for d in ds:
                    for q in range(HG):
                        aux_thunks.append(
                            lambda d=d, nb=nb, q=q: emit_final_mm(d, nb, q))

            def drain_aux(k, anchor=None):
                # anchor pins the aux matmul into this drain slot's position
                # in the PE stream - the scheduler's gap-filler otherwise
                # hoists thunks into earlier windows where their inputs are
                # still several microseconds from ready
                for _ in range(k):
                    if aux_thunks:
                        mm = aux_thunks.pop(0)()
                        if anchor is not None and mm is not None:
                            add_dep_helper(mm.ins, anchor.ins, sync=False,
                                           reason="pin aux to drain slot")

            # drains per j-slot for each pass (pass = 2*ib + p_): front-load
            # xh (consumed by AV from pass 0) and z2/cm, then pace the
            # remaining 128 aux matmuls so no pass runs dry (HAM re-throttles
            # the PE clock if it idles)
            DRAIN_SCHED = [8, 4, 4, 4, 2, 2, 2, 2]

            def make_tail(p_, ib, avs, ptt_last):
                # Two-part tail.  Head (next iteration, j==0): the last
                # j-pair's AV matmuls, the avsb copies that release the AV
                # PSUM banks, and a small DMA that spreads each denominator
                # row [1,512] to [64,8] so its reciprocal is ~150ns on DVE
                # instead of a 3.3us FIFO-hogging [1,512] InstReciprocal.
                # Norm (j==2): reciprocal, DMA back, broadcast, scale, add.
                icol = ib * 512
                avsbs, d64s = [], []

                def emit_head():
                    for hh in range(2):
                        h = 2 * p_ + hh
                        nc.tensor.matmul(
                            avs[hh][0:DH + 1, :],
                            lhsT=xh_aug[NCH - 1][:, 65 * h:65 * h + DH + 1],
                            rhs=ptt_last[:, 512 * hh:512 * hh + 512],
                            start=False, stop=True,
                        )
                    for hh in range(2):
                        avsb = tpool.tile([DH + 1, 512], F32, tag="avsb",
                                          name=f"avsb{p_}{ib}{hh}")
                        nc.vector.tensor_copy(avsb[:], avs[hh][0:DH + 1, :])
                        avsbs.append(avsb)
                        if ib != 3:
                            d64 = tpool.tile([64, 8], F32, tag="d64",
                                             name=f"d64_{p_}{ib}{hh}")
                            nc.sync.dma_start(d64[:], avsb[DH:DH + 1, :])
                            d64s.append(d64)

                def emit_norm():
                    for hh in range(2):
                        h = 2 * p_ + hh
                        rc = tpool.tile([1, 512], F32, tag="rc",
                                        name=f"rc{p_}{ib}{hh}")
                        if ib == 3:
                            # latency-critical last tails: direct reciprocal
                            # beats the two-DMA-hop partition-spread version
                            nc.vector.reciprocal(rc[:],
                                                 avsbs[hh][DH:DH + 1, :])
                        else:
                            d64r = tpool.tile([64, 8], F32, tag="d64r",
                                              name=f"d64r{p_}{ib}{hh}")
                            nc.vector.reciprocal(d64r[:], d64s[hh][:])
                            nc.sync.dma_start(rc[:], d64r[:])
                        bc = tpool.tile([64, 512], F32, tag="bc",
                                        name=f"bc{p_}{ib}{hh}")
                        nc.gpsimd.partition_broadcast(bc[:], rc[:])
                        tmp = tpool.tile([64, 512], F32, tag="tmp",
                                         name=f"tmp{p_}{ib}{hh}")
                        nc.vector.tensor_mul(tmp[:], avsbs[hh][0:DH, :], bc[:])
                        dst = cat4[h][:, icol:icol + 512]
                        nc.vector.tensor_add(dst, tmp[:], dst)
                return emit_head, emit_norm

            pending_tail = None
            # --- spatial attention: iterations (ib 512-block, pair),
            #     processing key chunks two at a time (j-pairs) ---
            for ib in range(4):
                for p_ in range(2):
                    # nb's cat4 block is complete once BOTH pairs' tails ran;
                    # the second pair's tails execute during (ib+1, p0), so
                    # finals(nb) join the aux queue at (ib+1, p1)
                    if p_ == 1 and ib >= 1:
                        queue_finals(ib - 1)
                    icol = ib * 512
                    ndrain = DRAIN_SCHED[2 * ib + p_]
                    avs = [psAV.tile([128, 512], F32, tag="av",
                                     name=f"av{p_}{ib}{q}") for q in range(2)]
                    pair_t = [None] * NCH
                    for j in range(NCH):  # key chunks
                        spt = psS.tile([128, 1024], F32, tag="S",
                                       name=f"S{p_}{ib}{j}")
                        s_anchor = None
                        for hh in range(2):
                            off = 64 * hh
                            s_anchor = nc.tensor.matmul(
                                spt[:, 512 * hh:512 * hh + 512],
                                lhsT=yhT[p_][off:off + 64,
                                             j * 128:(j + 1) * 128],
                                rhs=z1T[p_][off:off + 64, icol:icol + 512],
                                start=True, stop=True,
                            )
                        ptt = ptpool.tile([128, 1024], ATT, tag="pt",
                                          name=f"pt{p_}{ib}{j}")
                        nc.scalar.activation(ptt[:], spt[:], EXP, scale=SCALE)
                        pair_t[j] = ptt
                        if pending_tail is not None:
                            if j == 0:
                                pending_tail[0]()
                            elif j == 2:
                                pending_tail[1]()
                                pending_tail = None
                        drain_aux(ndrain, s_anchor)
                        if j > 0:
                            for hh in range(2):
                                h = 2 * p_ + hh
                                nc.tensor.matmul(
                                    avs[hh][0:DH + 1, :],
                                    lhsT=xh_aug[j - 1][:, 65 * h:65 * h + DH + 1],
                                    rhs=pair_t[j - 1][:, 512 * hh:512 * hh + 512],
                                    start=(j == 1), stop=False,
                                )
                    pending_tail = make_tail(p_, ib, avs, pair_t[NCH - 1])
            pending_tail[0]()
            pending_tail[1]()
            # warm-keeper: the last tail's normalization chain
            # (dma->recip->dma->broadcast->mul->add) leaves the PE idle just
            # long enough for HAM to re-throttle the clock to 1.2 GHz right
            # before the last 32 final-projection matmuls.  Keep it busy
            # with throwaway matmuls whose results are never read.
            warm = psaux.tile([128, 512], F32, tag="aux", name="warmk")
            for w in range(40):
                nc.tensor.matmul(
                    warm[:],
                    lhsT=yhT[0][0:128, 0:128],
                    rhs=z1T[0][0:128, 0:512],
                    start=(w == 0), stop=(w == 39),
                )
            queue_finals(3)
            drain_aux(len(aux_thunks))

    nc.compile()
    return nc


_NC_CACHE = {}


def _get_program():
    if "nc" not in _NC_CACHE:
        _NC_CACHE["nc"] = _build_program()
    return _NC_CACHE["nc"]


def _prep_input_maps(x, y, z, w_sa1, w_sa2, w_se1, w_se2, w_out):
    bf16 = lambda a: np.ascontiguousarray(
        np.asarray(a, dtype=np.float32).astype(ml_dtypes.bfloat16))
    # wB[p, k*CIN+o] = w[k*128+p, o]
    wblk = lambda w: w.reshape(KC, 128, CIN).transpose(1, 0, 2) \
                      .reshape(128, KC * CIN)
    maps = []
    for c in range(NCORES):
        b, g = divmod(c, G)
        sl = slice(g * CIN, (g + 1) * CIN)
        xT = np.asarray(x)[b].T  # [DIM, N]
        # xB[i*128+p, k*128+j] = xT[k*128+p, i*128+j]
        xBlk = xT.reshape(KC, 128, NCH, 128).transpose(2, 1, 0, 3) \
                 .reshape(N, DIM)
        maps.append({
            "xB": bf16(xBlk),
            "yT": bf16(np.asarray(y)[b].T),
            "zT": bf16(np.asarray(z)[b].T),
            "w_sa1": bf16(wblk(np.asarray(w_sa1)[:, sl])),
            "w_sa2": bf16(wblk(np.asarray(w_sa2)[:, sl])),
            "w_se1": bf16(wblk(np.asarray(w_se1)[:, sl])),
            "w_se2": bf16(wblk(np.asarray(w_se2)[:, sl])),
            "w_out": bf16(np.asarray(w_out)[sl, :]),
        })
    return maps


def run(inputs, trace=False, trace_kwargs=None):
    """Run on hardware; returns (full_output, BassKernelResults)."""
    nc = _get_program()
    in_maps = _prep_input_maps(
        inputs["x"], inputs["y"], inputs["z"],
        inputs["w_sa1"], inputs["w_sa2"], inputs["w_se1"], inputs["w_se2"],
        inputs["w_out"],
    )
    res = run_bass_kernel_spmd(
        nc, in_maps, list(range(NCORES)), trace=trace,
        trace_kwargs=trace_kwargs or {},
    )
    out = np.zeros((B, N, DIM), dtype=np.float32)
    for c in range(NCORES):
        b, _g = divmod(c, G)
        out[b] += np.asarray(res.results[c]["outT"]).astype(np.float32).T
    out += np.asarray(inputs["b_out"], dtype=np.float32)
    return out, res


def kernel(**inputs) -> np.ndarray:
    out, _ = run(inputs, trace=False)
    return out
